# revision 36
# baseline (speedup 1.0000x reference)
"""Head-sharded tensor-parallel attention (2 heads/core, 8 cores).

Each core computes QKV for its 2 heads over ALL active tokens of both
batches, full attention for those heads, and a partial output
``AO_c @ W_out[c's 128 inner dims, :]``.  The host sums the 8 partial
outputs (the unshard step of the W_out-row sharding) and scatters back
into masked positions.  Masked-out rows of the reference output are
exactly zero, so only active tokens are processed (gathered on host);
pad keys have x=0 -> k=0 -> exp(0)=1, corrected by subtracting the pad
count from the softmax denominator.

The exp stream on the scalar engine is the critical resource (~47us):
emission order hand-interleaves the K/Q projection chunks with the S+exp
groups they unlock, fills PE slack with V / the other batch's
projections / AV passes / the out-projection, and keeps psum within
budget (sps 8KB + av 4KB + ss 4KB).
"""

import math
from contextlib import ExitStack

import numpy as np
import ml_dtypes

import concourse.bass as bass
import concourse.mybir as mybir
import concourse.tile as tile
from concourse import bacc
from concourse.bass_utils import run_bass_kernel_spmd

P = 128
D = 1024          # model dim
HEADS = 16
DH = 64
VW = DH + 1       # v columns per head + ones column (softmax denominator)
SCALE = DH ** -0.5
N_CORES = 8
BF16 = mybir.dt.bfloat16
F32 = mybir.dt.float32


def _chunks(total, step):
    out = []
    o = 0
    while o < total:
        out.append((o, min(step, total - o)))
        o += step
    return out


def _build(T: int, mq0: int, mq1: int):
    """Per-core SPMD graph; T = padded key count per batch (mult of 128),
    mq0/mq1 = per-batch query counts (real tokens padded to 16).
    x^T pad-key columns beyond mq are zeroed on device, not transferred."""
    nkt = T // P
    NT = 2 * T
    MQ = (mq0, mq1)
    KCS = _chunks(T, 512)                          # key chunks (K^T free dim)
    QCS = [_chunks(mq0, 512), _chunks(mq1, 512)]   # query chunks per batch

    nc = bacc.Bacc(None, target_bir_lowering=False, num_devices=N_CORES)

    xt_in = nc.declare_dram_parameter("xt", [D, NT], BF16, isOutput=False)
    wqkv_in = nc.declare_dram_parameter("wqkv", [D, 384], BF16, isOutput=False)
    wout_in = nc.declare_dram_parameter("wout", [P, D], BF16, isOutput=False)
    npad_in = nc.declare_dram_parameter("npad", [1, 2], F32, isOutput=False)
    out_ext = nc.declare_dram_parameter("out", [NT, D], BF16, isOutput=True)

    with tile.TileContext(nc) as tc, ExitStack() as ctx:
        sb = ctx.enter_context(tc.tile_pool(name="sb", bufs=1))
        ps = ctx.enter_context(tc.tile_pool(name="ps", bufs=1, space="PSUM"))

        npad_sb = sb.tile([1, 2], F32, tag="npad", bufs=1, name="npad_sb")
        nc.sync.dma_start(npad_sb[:], npad_in[:])

        # HAM warm-up: dependency-free matmuls so the PE clock ramps while
        # the first DMAs land (short, so they don't delay real work).
        warm = sb.tile([P, 384], BF16, tag="warm", bufs=1, name="warm")
        nc.vector.memset(warm[:], 0.0)
        for i in range(8):
            wps = ps.tile([P, 512], F32, tag="ss", bufs=2, name=f"wps{i}")
            nc.tensor.matmul(wps[:, 0:384], warm[:, 0:P], warm[:],
                             start=True, stop=True, skip_group_check=True)

        # ---- input DMAs, round-robined over sequencers.
        seqs = [nc.sync, nc.scalar, nc.gpsimd]
        _n = [0]

        def dma(dst, src, seq=None):
            (seqs[_n[0] % len(seqs)] if seq is None else seq).dma_start(dst, src)
            _n[0] += 1

        wqkv_sb = []
        for kc in range(8):
            tw = sb.tile([P, 384], BF16, tag="wqkv", bufs=8, name=f"wqkv{kc}")
            dma(tw[:], wqkv_in[kc * P:(kc + 1) * P, :])
            wqkv_sb.append(tw)
        xt = [sb.tile([P, NT], BF16, tag="xt", bufs=8, name=f"xt{kc}")
              for kc in range(8)]
        # pad key columns must read as zero (k=0 -> exp(0)=1 -> npad fix);
        # zero them on device instead of transferring them.
        for b in range(2):
            if MQ[b] < T:
                for kc in range(8):
                    nc.vector.memset(xt[kc][:, b * T + MQ[b]:(b + 1) * T], 0.0)
        # one large DMA per (kc, batch): per-DMA fixed cost dominates small
        # chunks (~1.4us per 128KB observed), so fewer+bigger wins the ramp.
        for b in range(2):
            for kc in range(8):
                dma(xt[kc][:, b * T: b * T + MQ[b]],
                    xt_in[kc * P:(kc + 1) * P, b * T: b * T + MQ[b]])
        wout_sb = sb.tile([P, D], BF16, tag="wout", bufs=1, name="wout_sb")
        dma(wout_sb[:, 0:512], wout_in[:, 0:512])
        dma(wout_sb[:, 512:D], wout_in[:, 512:D])

        # ---- building blocks -------------------------------------------
        kf = [None, None]
        qt = [None, None]

        def kq_alloc(b):
            kf[b] = sb.tile([P, T], BF16, tag="kf", bufs=2, name=f"kf{b}")
            qt[b] = sb.tile([P, T], BF16, tag="qt", bufs=2, name=f"qt{b}")

        def kq_chunk(b, dst, col0, qo, qw):
            """One projection chunk: dst[:, qo:qo+qw] = (Wcol^T x)[:, chunk]."""
            pps = ps.tile([P, 512], F32, tag="ss", bufs=2,
                          name=f"pp{b}_{col0}_{qo}")
            for i in range(8):
                kc = (i + qo // 512) % 8
                nc.tensor.matmul(
                    pps[:, 0:qw],
                    wqkv_sb[kc][:, col0:col0 + P],
                    xt[kc][:, b * T + qo: b * T + qo + qw],
                    start=(i == 0), stop=(i == 7))
            nc.vector.tensor_copy(dst[:, qo:qo + qw], pps[:, 0:qw])

        def kq(b, ci):
            if ci < len(KCS):
                kq_chunk(b, kf[b], 128, *KCS[ci])
            if ci < len(QCS[b]):
                kq_chunk(b, qt[b], 0, *QCS[b][ci])

        vt = [[None] * nkt, [None] * nkt]

        def proj_v(b):
            for kt in range(nkt):
                t_ = sb.tile([P, 2 * VW], BF16, tag="vt", bufs=2 * nkt,
                             name=f"vt{b}_{kt}")
                nc.gpsimd.memset(
                    t_[:].rearrange("p (h c) -> p h c", c=VW)[:, :, DH:DH + 1], 1.0)
                vps = ps.tile([P, P], F32, tag="ss", bufs=2, name=f"vps{b}_{kt}")
                for i in range(8):
                    kc = (i + kt) % 8
                    nc.tensor.matmul(
                        vps[:],
                        xt[kc][:, b * T + kt * P: b * T + (kt + 1) * P],
                        wqkv_sb[kc][:, 256:384],
                        start=(i == 0), stop=(i == 7))
                nc.vector.tensor_copy(
                    t_[:].rearrange("p (h c) -> p h c", c=VW)[:, :, 0:DH],
                    vps[:].rearrange("p (h c) -> p h c", c=DH))
                vt[b][kt] = t_

        pt = {}          # (b, qci, kt) -> [128 keys, 1024] bf16 (2 heads)

        def s_exp(b, qci, kts):
            qo, qw = QCS[b][qci]
            for kt in kts:
                ptt = sb.tile([P, 1024], BF16, tag="pt", bufs=2 * nkt + 4,
                              name=f"pt{b}_{qci}_{kt}")
                pt[(b, qci, kt)] = ptt
                sps = ps.tile([P, 1024], F32, tag="sps", bufs=2,
                              name=f"sps{b}_{kt}_{qo}")
                for h in range(2):
                    nc.tensor.matmul(
                        sps[:, h * 512: h * 512 + qw],
                        kf[b][h * DH:(h + 1) * DH, kt * P:(kt + 1) * P],
                        qt[b][h * DH:(h + 1) * DH, qo:qo + qw],
                        start=True, stop=True, skip_group_check=True)
                nc.scalar.activation(
                    ptt[:].rearrange("p (u c) -> p u c", c=512)[:, 0:2, 0:qw],
                    sps[:].rearrange("p (u c) -> p u c", c=512)[:, 0:2, 0:qw],
                    mybir.ActivationFunctionType.Exp, scale=SCALE)

        aoT = [None, None]
        tmpb = [None, None]

        def ao_alloc(b):
            aoT[b] = sb.tile([P, T], BF16, tag="aoT", bufs=2, name=f"aoT{b}")
            tmpb[b] = sb.tile([DH, T], BF16, tag="tmpb", bufs=2, name=f"tmpb{b}")

        def av_norm(b, qci):
            """AV accumulation + normalize for one (batch, query chunk)."""
            qo, qw = QCS[b][qci]
            for h in range(2):
                avp = ps.tile([P, 512], F32, tag="av", bufs=2,
                              name=f"avp{b}_{h}_{qo}")
                for kt in range(nkt):
                    nc.tensor.matmul(
                        avp[0:VW, 0:qw],
                        vt[b][kt][:, h * VW:(h + 1) * VW],
                        pt[(b, qci, kt)][:, h * 512: h * 512 + qw],
                        start=(kt == 0), stop=(kt == nkt - 1),
                        skip_group_check=True)
                # normalize straight out of psum (no staging copy)
                den = sb.tile([1, 512], F32, tag="den", bufs=4,
                              name=f"den{b}_{h}_{qo}")
                nc.vector.tensor_scalar(den[:, 0:qw], avp[DH:DH + 1, 0:qw],
                                        npad_sb[0:1, b:b + 1], None,
                                        op0=mybir.AluOpType.subtract)
                rec = sb.tile([1, 512], F32, tag="rec", bufs=4,
                              name=f"rec{b}_{h}_{qo}")
                nc.vector.reciprocal_approx_fast(rec[:, 0:qw], den[:, 0:qw])
                fac = sb.tile([DH, 512], F32, tag="fac", bufs=2,
                              name=f"fac{b}_{h}_{qo}")
                nc.gpsimd.partition_broadcast(fac[:, 0:qw], rec[:, 0:qw])
                if h == 0:
                    nc.vector.tensor_tensor(aoT[b][0:DH, qo:qo + qw],
                                            avp[0:DH, 0:qw], fac[:, 0:qw],
                                            op=mybir.AluOpType.mult)
                else:
                    nc.vector.tensor_tensor(tmpb[b][:, qo:qo + qw],
                                            avp[0:DH, 0:qw], fac[:, 0:qw],
                                            op=mybir.AluOpType.mult)
                    # partition shift 0:64 -> 64:128 needs a DMA, not DVE
                    (nc.sync if qo % 1024 == 0 else nc.gpsimd).dma_start(
                        aoT[b][DH:P, qo:qo + qw], tmpb[b][:, qo:qo + qw])

        def out_proj(b, mts, split_copies):
            for mt in mts:
                pm = min(P, MQ[b] - mt * P)
                osb = sb.tile([P, D], BF16, tag="osb", bufs=3, name=f"osb{b}_{mt}")
                for nf in range(2):
                    # alternate psum tags -> 4 rotating banks, so matmul(mt+1)
                    # overlaps the evacuation copies of mt
                    ops = ps.tile([P, 512], F32, tag=("ss" if nf == 0 else "av"),
                                  bufs=2, name=f"op{b}_{mt}_{nf}")
                    nc.tensor.matmul(ops[0:pm, :],
                                     aoT[b][:, mt * P: mt * P + pm],
                                     wout_sb[:, nf * 512:(nf + 1) * 512],
                                     start=True, stop=True, skip_group_check=True)
                    if nf == 1 and split_copies:
                        # scalar engine's queue drains these after the exps
                        nc.scalar.activation(osb[0:pm, 512:D], ops[0:pm, :],
                                             mybir.ActivationFunctionType.Copy)
                    else:
                        nc.vector.tensor_copy(osb[0:pm, nf * 512:(nf + 1) * 512],
                                              ops[0:pm, :])
                # one DMA per tile: sequencer issue (~0.65us) dominates halves
                dma(out_ext[b * T + mt * P: b * T + mt * P + pm, :],
                    osb[0:pm, :], seq=(nc.sync if mt % 2 == 0 else nc.gpsimd))

        # ---- hand-interleaved schedule (program order = priority) ------
        kq_alloc(0)
        kq_alloc(1)
        ao_alloc(0)
        ao_alloc(1)
        nmt0 = math.ceil(MQ[0] / P)
        nmt1 = math.ceil(MQ[1] / P)
        kq(0, 0)
        s_exp(0, 0, range(0, 4))       # needs kf0/qt0 chunk 0 only
        kq(0, 1)
        s_exp(0, 0, range(4, 8))
        kq(0, 2)
        s_exp(0, 0, range(8, nkt))
        s_exp(0, 1, range(nkt))
        proj_v(0)                      # PE filler under the exp stream
        s_exp(0, 2, range(nkt))
        kq(1, 0)
        kq(1, 1)
        kq(1, 2)
        s_exp(1, 0, range(nkt))
        av_norm(0, 0)
        av_norm(0, 1)
        proj_v(1)
        av_norm(0, 2)
        out_proj(0, range(nmt0), split_copies=False)   # scalar = exps
        s_exp(1, 1, range(nkt))
        av_norm(1, 0)
        s_exp(1, 2, range(nkt))
        av_norm(1, 1)
        av_norm(1, 2)
        out_proj(1, range(nmt1), split_copies=True)

    nc.compile()
    return nc


_GRAPH_CACHE: dict = {}


def _get_graph(T: int, mq0: int, mq1: int):
    key = (T, mq0, mq1)
    if key not in _GRAPH_CACHE:
        _GRAPH_CACHE[key] = _build(T, mq0, mq1)
    return _GRAPH_CACHE[key]


def kernel(x, mask, W_qkv, W_out):
    x = np.asarray(x, dtype=np.float32)
    mask = np.asarray(mask, dtype=np.float32)
    W_qkv = np.asarray(W_qkv, dtype=np.float32)
    W_out = np.asarray(W_out, dtype=np.float32)
    b, n, d = x.shape
    assert (b, d) == (2, D) and W_qkv.shape == (D, 3 * D)

    idx = [np.nonzero(mask[i] > 0.5)[0] for i in range(b)]
    m = [len(ix) for ix in idx]
    nkt = max(1, math.ceil(max(m) / P))
    T = nkt * P
    mq = [min(T, max(16, math.ceil(mi / 16) * 16)) for mi in m]

    nc = _get_graph(T, mq[0], mq[1])

    bf16 = ml_dtypes.bfloat16
    xg = np.zeros((b, T, d), dtype=np.float32)
    for i in range(b):
        xg[i, :m[i]] = x[i][idx[i]]
    xt_all = np.ascontiguousarray(
        xg.reshape(b * T, d).transpose(1, 0)).astype(bf16)   # [D, 2T]
    npad = np.array([[T - m[0], T - m[1]]], dtype=np.float32)

    in_maps = []
    for c in range(N_CORES):
        cols = slice(c * P, (c + 1) * P)
        wqkv_c = np.ascontiguousarray(np.concatenate(
            [W_qkv[:, 0 * D:1 * D][:, cols],
             W_qkv[:, 1 * D:2 * D][:, cols],
             W_qkv[:, 2 * D:3 * D][:, cols]], axis=1)).astype(bf16)
        wout_c = np.ascontiguousarray(W_out[cols, :]).astype(bf16)
        in_maps.append({
            "xt": xt_all,
            "wqkv": wqkv_c,
            "wout": wout_c,
            "npad": npad,
        })

    res = run_bass_kernel_spmd(nc, in_maps, core_ids=list(range(N_CORES)))

    total = np.zeros((b * T, d), dtype=np.float32)
    for c in range(N_CORES):
        total += np.asarray(res.results[c]["out"], dtype=np.float32)

    out = np.zeros((b, n, d), dtype=np.float32)
    for i in range(b):
        out[i][idx[i]] = total[i * T: i * T + m[i]]
    return out
